# revision 8
# baseline (speedup 1.0000x reference)
"""Trainium2 Bass kernel for nn_CompositionalFFN (moe_routing).

Reference computation:
    W1 = sum_k softmax(top32(fc1_logits))_k * fc1_A[idx_k] @ fc1_B[idx_k]   (1024, 4096)
    W2 = sum_k softmax(top32(fc2_logits))_k * fc2_A[idx_k] @ fc2_B[idx_k]   (4096, 1024)
    out = gelu_tanh(x @ W1) @ W2                                            (2, 1024, 1024)

Sharding strategy (8 cores): shard the FFN hidden axis F=4096 into 8 slices
of 512.  Each core composes its own F-slice of W1 and W2 from the 32
selected primitives (gathered from HBM - only 32/256 primitives are ever
read, which is the memory-regime win), computes hT = gelu(W1s^T @ x^T) for
all tokens, and the partial product outT_c = W2s^T @ hT.  The full output is
the sum over cores of the partials (unsharded on host).  No inter-core
communication is required.

Everything is computed in "transposed token space" so no on-device
transposes are needed anywhere:
    - host uploads x^T (D, T) and A-banks pre-transposed to (r, d_in) rows
    - W1 slice is built in (D, F) layout = lhsT for the first GEMM
    - W2 slice is built in (F, D) layout = lhsT for the second GEMM
The top-k selection + softmax (a 256-element op) runs on host; the selected
primitive indices are baked into the DMA program built for this call, and
the softmax weights are applied on-device via per-partition tensor_scalar
multiplies.
"""

import os
import sys

import numpy as np

for _p in ("/opt/trn_rl_repo", "/root/.axon_site/_ro/trn_rl_repo"):
    if os.path.isdir(_p) and _p not in sys.path:
        sys.path.append(_p)

import concourse.bass as bass
import concourse.mybir as mybir
import concourse.tile as tile
from concourse import bacc
from concourse.bass_utils import run_bass_kernel_spmd

# Problem shapes (hardcoded per contract)
B, S, D, F, P, R = 2, 1024, 1024, 4096, 256, 64
T = B * S                 # 2048 tokens
TOP_K = 32
N_CORES = 8
FSL = F // N_CORES        # 512: F-slice per core
NPAIR = TOP_K // 2        # 16: primitives are processed in pairs (2*64 = 128 partitions)

F32 = mybir.dt.float32
F32R = mybir.dt.float32r


def _topk_softmax(logits: np.ndarray):
    """Match jax.lax.top_k: descending values, ties broken by lower index."""
    idx = np.argsort(-logits, kind="stable")[:TOP_K]
    vals = logits[idx].astype(np.float64)
    e = np.exp(vals - vals.max())
    w = (e / e.sum()).astype(np.float32)
    return idx.astype(np.int64), w


def _build_program(mm_dt=F32R, act=None):
    """Build the single-core Bass/Tile program (identical on all 8 cores)."""
    if act is None:
        act = mybir.ActivationFunctionType.Gelu_apprx_tanh
    nc = bacc.Bacc("TRN2", target_bir_lowering=False, debug=False,
                   num_devices=N_CORES)

    # DRAM I/O (per-core shards; names are the in_map keys).  Tensors that
    # feed the PE array are declared in the matmul dtype (float32r = fp32
    # bits, PE fast mode) so the verifier sees properly-typed producers.
    a1t = nc.dram_tensor("a1t", [P * R, D], mm_dt, kind="ExternalInput")    # A1^T rows (256*64, 1024)
    b1 = nc.dram_tensor("b1", [P * R, FSL], mm_dt, kind="ExternalInput")    # B1 F-slice rows
    a2t = nc.dram_tensor("a2t", [P * R, FSL], mm_dt, kind="ExternalInput")  # A2^T F-slice rows
    b2 = nc.dram_tensor("b2", [P * R, D], mm_dt, kind="ExternalInput")      # B2 rows
    xt = nc.dram_tensor("xt", [D, T], mm_dt, kind="ExternalInput")          # x^T
    wtab = nc.dram_tensor("wtab", [128, 2 * NPAIR], F32, kind="ExternalInput")
    outp = nc.dram_tensor("outp", [D, T], F32, kind="ExternalOutput")       # partial out^T

    with tile.TileContext(nc) as tc:
        with (
            tc.tile_pool(name="const", bufs=1) as const_pool,
            tc.tile_pool(name="xt", bufs=8) as xt_pool,
            tc.tile_pool(name="w1", bufs=8) as w1_pool,
            tc.tile_pool(name="w2", bufs=4) as w2_pool,
            tc.tile_pool(name="ht", bufs=4) as ht_pool,
            tc.tile_pool(name="ga1", bufs=3) as ga1_pool,
            tc.tile_pool(name="gb1", bufs=3) as gb1_pool,
            tc.tile_pool(name="ga2", bufs=4) as ga2_pool,
            tc.tile_pool(name="gb2", bufs=4) as gb2_pool,
            tc.tile_pool(name="ost", bufs=4) as o_pool,
            tc.tile_pool(name="ps", bufs=8, space="PSUM") as ps_pool,
        ):
            wtab_t = const_pool.tile([128, 2 * NPAIR], F32)
            nc.sync.dma_start(out=wtab_t[:], in_=wtab[:])

            # x^T resident tiles: 8 x (128, 2048)
            xt_tiles = []
            for d in range(D // 128):
                xtt = xt_pool.tile([128, T], mm_dt, tag="xt")
                nc.sync.dma_start(out=xtt[:], in_=xt[d * 128:(d + 1) * 128, :])
                xt_tiles.append(xtt)

            # ---- Phase 1: compose W1 slice (D, FSL) in PSUM banks 0..7 ----
            w1_psum = [ps_pool.tile([128, 512], F32, space="PSUM", tag="ps",
                                    name=f"w1ps{d}") for d in range(8)]
            for j in range(NPAIR):
                at = ga1_pool.tile([128, D], mm_dt, tag="ga1")
                bt = gb1_pool.tile([128, FSL], mm_dt, tag="gb1")
                # gather pair (k1, k2): rows k*64..k*64+64 of the banks
                for h in range(2):
                    k = PAIRS1[2 * j + h]
                    nc.sync.dma_start(out=at[h * 64:(h + 1) * 64, :],
                                      in_=a1t[k * R:(k + 1) * R, :])
                    nc.sync.dma_start(out=bt[h * 64:(h + 1) * 64, :],
                                      in_=b1[k * R:(k + 1) * R, :])
                # scale rhs rows by the pair's softmax weights (in place)
                nc.vector.tensor_scalar_mul(bt[:], bt[:], wtab_t[:, j:j + 1])
                for d in range(8):
                    nc.tensor.matmul(
                        w1_psum[d][:],
                        lhsT=at[:, d * 128:(d + 1) * 128],
                        rhs=bt[:],
                        start=(j == 0), stop=(j == NPAIR - 1),
                    )
            w1_tiles = []
            for d in range(8):
                w1t = w1_pool.tile([128, FSL], mm_dt, tag="w1")
                nc.vector.tensor_copy(out=w1t[:], in_=w1_psum[d][:])
                w1_tiles.append(w1t)

            # ---- Phase 2: hT = gelu(W1s^T @ x^T), (FSL, T) ----
            ht_tiles = []
            for f in range(FSL // 128):
                htt = ht_pool.tile([128, T], mm_dt, tag="ht")
                for t in range(T // 512):
                    ps = ps_pool.tile([128, 512], F32, space="PSUM", tag="ps")
                    for d in range(8):
                        nc.tensor.matmul(
                            ps[:],
                            lhsT=w1_tiles[d][:, f * 128:(f + 1) * 128],
                            rhs=xt_tiles[d][:, t * 512:(t + 1) * 512],
                            start=(d == 0), stop=(d == 7),
                        )
                    nc.scalar.activation(
                        htt[:, t * 512:(t + 1) * 512], ps[:], act,
                    )
                ht_tiles.append(htt)

            # ---- Phase 3: compose W2 slice (FSL, D) ----
            w2_psum = [ps_pool.tile([128, 512], F32, space="PSUM", tag="ps",
                                    name=f"w2ps{i}") for i in range(8)]
            for j in range(NPAIR):
                at = ga2_pool.tile([128, FSL], mm_dt, tag="ga2")
                bt = gb2_pool.tile([128, D], mm_dt, tag="gb2")
                for h in range(2):
                    k = PAIRS2[2 * j + h]
                    nc.sync.dma_start(out=at[h * 64:(h + 1) * 64, :],
                                      in_=a2t[k * R:(k + 1) * R, :])
                    nc.sync.dma_start(out=bt[h * 64:(h + 1) * 64, :],
                                      in_=b2[k * R:(k + 1) * R, :])
                nc.vector.tensor_scalar_mul(at[:], at[:], wtab_t[:, NPAIR + j:NPAIR + j + 1])
                for f in range(4):
                    for n in range(2):
                        nc.tensor.matmul(
                            w2_psum[f * 2 + n][:],
                            lhsT=at[:, f * 128:(f + 1) * 128],
                            rhs=bt[:, n * 512:(n + 1) * 512],
                            start=(j == 0), stop=(j == NPAIR - 1),
                        )
            w2_tiles = []
            for f in range(4):
                w2t = w2_pool.tile([128, D], mm_dt, tag="w2")
                for n in range(2):
                    nc.vector.tensor_copy(out=w2t[:, n * 512:(n + 1) * 512],
                                          in_=w2_psum[f * 2 + n][:])
                w2_tiles.append(w2t)

            # ---- Phase 4: outT partial = W2s^T @ hT, (D, T) ----
            for dd in range(D // 128):
                for t in range(T // 512):
                    ps = ps_pool.tile([128, 512], F32, space="PSUM", tag="ps")
                    for f in range(4):
                        nc.tensor.matmul(
                            ps[:],
                            lhsT=w2_tiles[f][:, dd * 128:(dd + 1) * 128],
                            rhs=ht_tiles[f][:, t * 512:(t + 1) * 512],
                            start=(f == 0), stop=(f == 3),
                        )
                    ot = o_pool.tile([128, 512], F32, tag="ost")
                    nc.vector.tensor_copy(out=ot[:], in_=ps[:])
                    nc.sync.dma_start(
                        out=outp[dd * 128:(dd + 1) * 128, t * 512:(t + 1) * 512],
                        in_=ot[:])
    nc.compile()
    return nc


# Pair index tables get baked into the DMA program; set per call.
PAIRS1 = list(range(TOP_K))
PAIRS2 = list(range(TOP_K))

# (nc, in_maps) of the last kernel() call, for re-running with trace.
LAST_RUN_STATE = None


def kernel(x, fc1_A, fc1_B, fc2_A, fc2_B, fc1_logits, fc2_logits):
    global PAIRS1, PAIRS2
    x = np.ascontiguousarray(np.asarray(x, dtype=np.float32))
    fc1_A = np.asarray(fc1_A, dtype=np.float32)
    fc1_B = np.asarray(fc1_B, dtype=np.float32)
    fc2_A = np.asarray(fc2_A, dtype=np.float32)
    fc2_B = np.asarray(fc2_B, dtype=np.float32)

    idx1, w1 = _topk_softmax(np.asarray(fc1_logits, dtype=np.float32))
    idx2, w2 = _topk_softmax(np.asarray(fc2_logits, dtype=np.float32))
    PAIRS1 = [int(k) for k in idx1]
    PAIRS2 = [int(k) for k in idx2]

    # weight table: column j = per-partition scale for pair j (64x w_{2j}, 64x w_{2j+1});
    # columns 0..15 layer 1, 16..31 layer 2
    wtab = np.zeros((128, 2 * NPAIR), np.float32)
    for j in range(NPAIR):
        wtab[:64, j] = w1[2 * j]
        wtab[64:, j] = w1[2 * j + 1]
        wtab[:64, NPAIR + j] = w2[2 * j]
        wtab[64:, NPAIR + j] = w2[2 * j + 1]

    # Host-side layout prep (sharding): transposed A banks, x^T, per-core F-slices
    a1t_np = np.ascontiguousarray(fc1_A.transpose(0, 2, 1)).reshape(P * R, D)
    b2_np = np.ascontiguousarray(fc2_B).reshape(P * R, D)
    xt_np = np.ascontiguousarray(x.reshape(T, D).T)

    in_maps = []
    for c in range(N_CORES):
        fsl = slice(c * FSL, (c + 1) * FSL)
        b1_np = np.ascontiguousarray(fc1_B[:, :, fsl]).reshape(P * R, FSL)
        a2t_np = np.ascontiguousarray(fc2_A[:, fsl, :].transpose(0, 2, 1)).reshape(P * R, FSL)
        in_maps.append({
            "a1t": a1t_np, "b1": b1_np, "a2t": a2t_np, "b2": b2_np,
            "xt": xt_np, "wtab": wtab,
        })

    nc = _build_program()
    global LAST_RUN_STATE
    LAST_RUN_STATE = (nc, in_maps)
    res = run_bass_kernel_spmd(nc, in_maps, core_ids=list(range(N_CORES)))

    outT = np.zeros((D, T), np.float64)
    for r in res.results:
        outT += r["outp"].astype(np.float64)
    return np.ascontiguousarray(outT.T.astype(np.float32)).reshape(B, S, D)


if __name__ == "__main__":
    rng = np.random.default_rng(0)
    inputs = {
        "x": rng.standard_normal((B, S, D), dtype=np.float32),
        "fc1_A": rng.standard_normal((P, D, R), dtype=np.float32) * 0.02,
        "fc1_B": rng.standard_normal((P, R, F), dtype=np.float32) * 0.02,
        "fc2_A": rng.standard_normal((P, F, R), dtype=np.float32) * 0.02,
        "fc2_B": rng.standard_normal((P, R, D), dtype=np.float32) * 0.02,
        "fc1_logits": rng.standard_normal(P, dtype=np.float32),
        "fc2_logits": rng.standard_normal(P, dtype=np.float32),
    }
    out = kernel(**inputs)
    print(out.shape, out.dtype, np.abs(out).max())
